# revision 12
# baseline (speedup 1.0000x reference)
"""Trainium2 Bass kernel for a ResNet BasicBlock with ternary 3x3 convs.

  y = relu(x + bn2(conv2(relu(bn1(conv1(x))))))

x: [64, 256, 32, 32] f32.  Data-parallel over batch: 8 images per core on
8 NeuronCores.  Each 3x3 conv is computed as 9 shifted matmuls (one per tap)
accumulated in PSUM, contracting over input channels (2 blocks of 128).
Activations live in SBUF in a zero-padded 34x34 per-image layout so every
tap is a strided window read — no edge fixups.  Matmuls run in float32r
(full-rate fp32 mode on TRN2).  BN is folded into a per-channel scale+bias
applied by the scalar engine fused with ReLU; the residual add is fused
into one vector-engine scalar_tensor_tensor op.
"""

import os
from contextlib import ExitStack

import numpy as np

import concourse.bass as bass
import concourse.tile as tile
from concourse import bacc, mybir
from concourse.bass_utils import run_bass_kernel_spmd

F32 = mybir.dt.float32
F32R = mybir.dt.float32r

N_CORES = 8
N_FULL = 64          # full batch
C = 256              # channels
H = W = 32
P = 128              # partitions
CB = C // P          # channel blocks (2)
TAPS = 9
HP = H + 2           # padded height (34)
WP = W + 2           # padded width (34)
PAD = HP * WP        # padded image size (1156)
HALF = (H // 2) * W  # 512 output elements per psum tile
NIMG = N_FULL // N_CORES  # images per core (8)

# matmul dtype mode: "f32r" (TF32-rate, near-fp32 precision) or "f32" (exact, 1/4 rate)
MM_MODE = os.environ.get("BB_MM_MODE", "f32r")
_MM_DT = {"f32r": F32R, "f32": F32}[MM_MODE]

XR = 3  # x-pad ring depth (images in flight for conv1 input + residual)
HR = 2  # h-pad ring depth


def _emit_conv(nc, ws, src, ps):
    """Emit the 72 matmuls of one 3x3 conv for one image.

    ws: per-cib weight tiles [128ci, TAPS*CB*128co]
    src: padded activation tile [128, CB, PAD]
    ps: dict (cob, half) -> PSUM tile [128, HALF]
    """
    for cib in range(CB):
        s3 = src[:, cib].rearrange("p (r c) -> p r c", c=WP)
        for tap in range(TAPS):
            dy, dx = divmod(tap, 3)
            for cob in range(CB):
                w_ap = ws[cib][:, (tap * CB + cob) * P : (tap * CB + cob + 1) * P]
                for half in range(2):
                    rhs = s3[:, half * 16 + dy : half * 16 + dy + 16, dx : dx + 32]
                    nc.tensor.matmul(
                        ps[cob, half][:],
                        w_ap.bitcast(_MM_DT),
                        rhs.bitcast(_MM_DT),
                        start=(cib == 0 and tap == 0),
                        stop=(cib == CB - 1 and tap == TAPS - 1),
                    )


def build(nimg: int = NIMG) -> bacc.Bacc:
    nc = bacc.Bacc("TRN2", target_bir_lowering=False, debug=False, enable_asserts=True)

    x_d = nc.dram_tensor("x", [nimg, C, H, W], F32, kind="ExternalInput")
    w1_d = nc.dram_tensor("w1t", [CB, P, TAPS * CB * P], F32, kind="ExternalInput")
    w2_d = nc.dram_tensor("w2t", [CB, P, TAPS * CB * P], F32, kind="ExternalInput")
    # bnv free layout: vec*CB + cob for vec in (inv1, b1', inv2, b2')
    bn_d = nc.dram_tensor("bnv", [P, 4 * CB], F32, kind="ExternalInput")
    y_d = nc.dram_tensor("y", [nimg, C, H, W], F32, kind="ExternalOutput")

    with tile.TileContext(nc) as tc, ExitStack() as ctx:
        wpool = ctx.enter_context(tc.tile_pool(name="weights", bufs=1))
        xpool = ctx.enter_context(tc.tile_pool(name="xpad", bufs=1))
        hpool = ctx.enter_context(tc.tile_pool(name="hpad", bufs=1))
        pspool = ctx.enter_context(tc.tile_pool(name="psum", bufs=8, space="PSUM"))
        respool = ctx.enter_context(tc.tile_pool(name="res", bufs=4))
        opool = ctx.enter_context(tc.tile_pool(name="out", bufs=3))

        # ---- constants: weights + folded BN vectors ----
        w1_s = []
        w2_s = []
        for cib in range(CB):
            t1 = wpool.tile([P, TAPS * CB * P], F32, tag=f"w1_{cib}", name=f"w1_{cib}")
            nc.sync.dma_start(t1[:].bitcast(_MM_DT), w1_d[cib].bitcast(_MM_DT))
            w1_s.append(t1)
            t2 = wpool.tile([P, TAPS * CB * P], F32, tag=f"w2_{cib}", name=f"w2_{cib}")
            nc.sync.dma_start(t2[:].bitcast(_MM_DT), w2_d[cib].bitcast(_MM_DT))
            w2_s.append(t2)
        bn_s = wpool.tile([P, 4 * CB], F32, tag="bn", name="bn_s")
        nc.sync.dma_start(bn_s[:], bn_d[:])

        def bnv(vec: int, cob: int):
            return bn_s[:, vec * CB + cob : vec * CB + cob + 1]

        # ---- persistent padded rings (pads stay zero forever) ----
        xslots = [xpool.tile([P, CB, PAD], F32, tag=f"xp{i}", name=f"xp{i}") for i in range(XR)]
        hslots = [hpool.tile([P, CB, PAD], F32, tag=f"hp{i}", name=f"hp{i}") for i in range(HR)]
        for s in xslots + hslots:
            nc.vector.memset(s[:], 0.0)

        def load_x(n):
            dst = xslots[n % XR]
            for cib in range(CB):
                d3 = dst[:, cib].rearrange("p (r c) -> p r c", c=WP)
                nc.sync.dma_start(
                    d3[:, 1 : H + 1, 1 : W + 1].bitcast(_MM_DT),
                    x_d[n, cib * P : (cib + 1) * P].bitcast(_MM_DT),
                )

        def conv1(n):
            ps = {}
            for cob in range(CB):
                for half in range(2):
                    ps[cob, half] = pspool.tile(
                        [P, HALF], F32, tag="ps", name=f"ps1_{n}_{cob}_{half}"
                    )
            _emit_conv(nc, w1_s, xslots[n % XR], ps)
            return ps

        def bn1_relu(n, ps):
            hdst = hslots[n % HR]
            for cob in range(CB):
                h3 = hdst[:, cob].rearrange("p (r c) -> p r c", c=WP)
                for half in range(2):
                    pv = ps[cob, half].rearrange("p (r c) -> p r c", c=W)
                    nc.scalar.activation(
                        h3[:, half * 16 + 1 : half * 16 + 17, 1 : W + 1].bitcast(
                            _MM_DT
                        ),
                        pv[:],
                        mybir.ActivationFunctionType.Relu,
                        bias=bnv(1, cob),
                        scale=bnv(0, cob),
                    )

        def conv2(n):
            ps = {}
            for cob in range(CB):
                for half in range(2):
                    ps[cob, half] = pspool.tile(
                        [P, HALF], F32, tag="ps", name=f"ps2_{n}_{cob}_{half}"
                    )
            _emit_conv(nc, w2_s, hslots[n % HR], ps)
            return ps

        def finish(n, ps):
            xsrc = xslots[n % XR]
            for cob in range(CB):
                ot = opool.tile([P, H * W], F32, tag="ot", name=f"ot_{n}_{cob}")
                x3 = xsrc[:, cob].rearrange("p (r c) -> p r c", c=WP)
                for half in range(2):
                    pv = ps[cob, half].rearrange("p (r c) -> p r c", c=W)
                    res = respool.tile([P, HALF], F32, tag="res", name=f"res_{n}_{cob}_{half}")
                    rv = res.rearrange("p (r c) -> p r c", c=W)
                    # res = conv2*inv2 + x   (vector engine, fused)
                    nc.vector.scalar_tensor_tensor(
                        rv[:],
                        pv[:],
                        bnv(2, cob),
                        x3[:, half * 16 + 1 : half * 16 + 17, 1 : W + 1],
                        op0=mybir.AluOpType.mult,
                        op1=mybir.AluOpType.add,
                    )
                    # out = relu(res + b2')   (scalar engine)
                    nc.scalar.activation(
                        ot[:, half * HALF : (half + 1) * HALF],
                        res[:],
                        mybir.ActivationFunctionType.Relu,
                        bias=bnv(3, cob),
                        scale=1.0,
                    )
                nc.sync.dma_start(
                    y_d[n, cob * P : (cob + 1) * P].rearrange("c h w -> c (h w)"),
                    ot[:],
                )

        # ---- software-pipelined emission ----
        # PE program order: conv1(0), conv1(1), conv2(0), conv1(2), conv2(1), ...
        # so the tensor engine always has conv1(n+1) to run while bn1(n)
        # completes on the scalar engine.
        for n in range(min(2, nimg)):
            load_x(n)
        ps1 = {0: conv1(0)} if nimg > 0 else {}
        for n in range(nimg):
            if n + 1 < nimg:
                bn1_relu(n, ps1.pop(n))
                ps1[n + 1] = conv1(n + 1)
            else:
                bn1_relu(n, ps1.pop(n))
            ps2 = conv2(n)
            finish(n, ps2)
            if n + 2 < nimg:
                load_x(n + 2)

    nc.compile()
    return nc


_NC_CACHE: dict = {}


def _get_nc(nimg: int = NIMG):
    if nimg not in _NC_CACHE:
        _NC_CACHE[nimg] = build(nimg)
    return _NC_CACHE[nimg]


def _prep_host(w1, g1, b1, rm1, rv1, w2, g2, b2, rm2, rv2):
    eps = 1e-5
    f = np.float32
    inv1 = (np.asarray(g1, f) / np.sqrt(np.asarray(rv1, f) + eps)).astype(f)
    b1p = (np.asarray(b1, f) - np.asarray(rm1, f) * inv1).astype(f)
    inv2 = (np.asarray(g2, f) / np.sqrt(np.asarray(rv2, f) + eps)).astype(f)
    b2p = (np.asarray(b2, f) - np.asarray(rm2, f) * inv2).astype(f)
    bnv = np.zeros((P, 4 * CB), f)
    for vi, v in enumerate([inv1, b1p, inv2, b2p]):
        for cob in range(CB):
            bnv[:, vi * CB + cob] = v[cob * P : (cob + 1) * P]

    def wt(w):
        w = np.asarray(w, f).reshape(CB, P, CB, P, 3, 3)  # [cob, co, cib, ci, ky, kx]
        w = w.transpose(2, 3, 4, 5, 0, 1)                 # [cib, ci, ky, kx, cob, co]
        return np.ascontiguousarray(w.reshape(CB, P, TAPS * CB * P))

    return wt(w1), wt(w2), bnv


def make_in_maps(x, w1, g1, b1, rm1, rv1, w2, g2, b2, rm2, rv2):
    x = np.asarray(x, np.float32)
    w1t, w2t, bnv = _prep_host(w1, g1, b1, rm1, rv1, w2, g2, b2, rm2, rv2)
    return [
        {
            "x": np.ascontiguousarray(x[c * NIMG : (c + 1) * NIMG]),
            "w1t": w1t,
            "w2t": w2t,
            "bnv": bnv,
        }
        for c in range(N_CORES)
    ]


def kernel(x, w1, g1, b1, rm1, rv1, w2, g2, b2, rm2, rv2):
    nc = _get_nc()
    in_maps = make_in_maps(x, w1, g1, b1, rm1, rv1, w2, g2, b2, rm2, rv2)
    res = run_bass_kernel_spmd(nc, in_maps, list(range(N_CORES)))
    return np.ascontiguousarray(
        np.concatenate([res.results[c]["y"] for c in range(N_CORES)], axis=0)
    )


# revision 13
# speedup vs baseline: 1.0356x; 1.0356x over previous
"""Trainium2 Bass kernel for a ResNet BasicBlock with ternary 3x3 convs.

  y = relu(x + bn2(conv2(relu(bn1(conv1(x))))))

x: [64, 256, 32, 32] f32.  Data-parallel over batch: 8 images per core on
8 NeuronCores.  Each 3x3 conv is computed as 9 shifted matmuls (one per tap)
accumulated in PSUM, contracting over input channels (2 blocks of 128).
Activations live in SBUF in a zero-padded 34x34 per-image layout so every
tap is a strided window read — no edge fixups.  x is pre-padded on the host
so its loads are contiguous DMAs.  Matmuls run in float32r (TF32-rate fp32
mode on TRN2; the ternary weights are exact in it).  BN is folded into a
per-channel scale+bias applied by the scalar engine fused with ReLU; the
residual add is fused into one vector-engine scalar_tensor_tensor op.
"""

import os
from contextlib import ExitStack

import numpy as np

import concourse.bass as bass
import concourse.tile as tile
from concourse import bacc, mybir
from concourse.bass_utils import run_bass_kernel_spmd

F32 = mybir.dt.float32
F32R = mybir.dt.float32r

N_CORES = 8
N_FULL = 64          # full batch
C = 256              # channels
H = W = 32
P = 128              # partitions
CB = C // P          # channel blocks (2)
TAPS = 9
HP = H + 2           # padded height (34)
WP = W + 2           # padded width (34)
PAD = HP * WP        # padded image size (1156)
HALF = (H // 2) * W  # 512 output elements per psum tile
NIMG = N_FULL // N_CORES  # images per core (8)

# matmul dtype mode: "f32r" (TF32-rate, near-fp32 precision) or "f32" (exact, 1/4 rate)
MM_MODE = os.environ.get("BB_MM_MODE", "f32r")
_MM_DT = {"f32r": F32R, "f32": F32}[MM_MODE]

XR = 3  # x tiles in flight (conv1 input + residual)
HR = 2  # h-pad ring depth


def _emit_conv(nc, ws, src, ps):
    """Emit the 72 matmuls of one 3x3 conv for one image.

    ws: per-cib weight tiles [128ci, TAPS*CB*128co]
    src: padded activation tile [128, CB, PAD]
    ps: dict (cob, half) -> PSUM tile [128, HALF]
    """
    for cib in range(CB):
        s3 = src[:, cib].rearrange("p (r c) -> p r c", c=WP)
        for tap in range(TAPS):
            dy, dx = divmod(tap, 3)
            for cob in range(CB):
                w_ap = ws[cib][:, (tap * CB + cob) * P : (tap * CB + cob + 1) * P]
                for half in range(2):
                    rhs = s3[:, half * 16 + dy : half * 16 + dy + 16, dx : dx + 32]
                    nc.tensor.matmul(
                        ps[cob, half][:],
                        w_ap.bitcast(_MM_DT),
                        rhs.bitcast(_MM_DT),
                        start=(cib == 0 and tap == 0),
                        stop=(cib == CB - 1 and tap == TAPS - 1),
                    )


def build(nimg: int = NIMG) -> bacc.Bacc:
    nc = bacc.Bacc("TRN2", target_bir_lowering=False, debug=False, enable_asserts=True)

    # x arrives host-pre-padded: [nimg, CB, 128, 34*34] with zero borders
    x_d = nc.dram_tensor("xp", [nimg, CB, P, PAD], F32, kind="ExternalInput")
    w1_d = nc.dram_tensor("w1t", [CB, P, TAPS * CB * P], F32, kind="ExternalInput")
    w2_d = nc.dram_tensor("w2t", [CB, P, TAPS * CB * P], F32, kind="ExternalInput")
    # bnv free layout: vec*CB + cob for vec in (inv1, b1', inv2, b2')
    bn_d = nc.dram_tensor("bnv", [P, 4 * CB], F32, kind="ExternalInput")
    y_d = nc.dram_tensor("y", [nimg, C, H, W], F32, kind="ExternalOutput")

    with tile.TileContext(nc) as tc, ExitStack() as ctx:
        wpool = ctx.enter_context(tc.tile_pool(name="weights", bufs=1))
        xpool = ctx.enter_context(tc.tile_pool(name="xpad", bufs=XR))
        hpool = ctx.enter_context(tc.tile_pool(name="hpad", bufs=1))
        pspool = ctx.enter_context(tc.tile_pool(name="psum", bufs=8, space="PSUM"))
        respool = ctx.enter_context(tc.tile_pool(name="res", bufs=4))
        opool = ctx.enter_context(tc.tile_pool(name="out", bufs=3))

        # ---- constants: weights + folded BN vectors ----
        # Weight/bn loads go on the scalar engine's HWDGE queue so they run in
        # parallel with the x loads on the sync engine's queue.  conv1 only
        # needs w1, so load it first.
        w1_s = []
        w2_s = []
        for cib in range(CB):
            t1 = wpool.tile([P, TAPS * CB * P], F32, tag=f"w1_{cib}", name=f"w1_{cib}")
            nc.scalar.dma_start(t1[:].bitcast(_MM_DT), w1_d[cib].bitcast(_MM_DT))
            w1_s.append(t1)
        bn_s = wpool.tile([P, 4 * CB], F32, tag="bn", name="bn_s")
        nc.scalar.dma_start(bn_s[:], bn_d[:])
        for cib in range(CB):
            t2 = wpool.tile([P, TAPS * CB * P], F32, tag=f"w2_{cib}", name=f"w2_{cib}")
            nc.scalar.dma_start(t2[:].bitcast(_MM_DT), w2_d[cib].bitcast(_MM_DT))
            w2_s.append(t2)

        def bnv(vec: int, cob: int):
            return bn_s[:, vec * CB + cob : vec * CB + cob + 1]

        # ---- persistent padded h ring (pads zeroed once, stay zero) ----
        hslots = [hpool.tile([P, CB, PAD], F32, tag=f"hp{i}", name=f"hp{i}") for i in range(HR)]
        for s in hslots:
            for cib in range(CB):
                h3 = s[:, cib].rearrange("p (r c) -> p r c", c=WP)
                nc.vector.memset(h3[:, 0 : HP : HP - 1, :], 0.0)       # top+bottom rows
                nc.vector.memset(h3[:, 1 : HP - 1, 0 : WP : WP - 1], 0.0)  # side cols

        xtiles = {}

        def load_x(n):
            t = xpool.tile([P, CB, PAD], F32, tag="xp", name=f"xt_{n}")
            for cib in range(CB):
                nc.sync.dma_start(
                    t[:, cib].bitcast(_MM_DT), x_d[n, cib].bitcast(_MM_DT)
                )
            xtiles[n] = t

        def conv1(n):
            ps = {}
            for cob in range(CB):
                for half in range(2):
                    ps[cob, half] = pspool.tile(
                        [P, HALF], F32, tag="ps", name=f"ps1_{n}_{cob}_{half}"
                    )
            _emit_conv(nc, w1_s, xtiles[n], ps)
            return ps

        def bn1_relu(n, ps):
            hdst = hslots[n % HR]
            for cob in range(CB):
                h3 = hdst[:, cob].rearrange("p (r c) -> p r c", c=WP)
                for half in range(2):
                    pv = ps[cob, half].rearrange("p (r c) -> p r c", c=W)
                    nc.scalar.activation(
                        h3[:, half * 16 + 1 : half * 16 + 17, 1 : W + 1].bitcast(
                            _MM_DT
                        ),
                        pv[:],
                        mybir.ActivationFunctionType.Relu,
                        bias=bnv(1, cob),
                        scale=bnv(0, cob),
                    )

        def conv2(n):
            ps = {}
            for cob in range(CB):
                for half in range(2):
                    ps[cob, half] = pspool.tile(
                        [P, HALF], F32, tag="ps", name=f"ps2_{n}_{cob}_{half}"
                    )
            _emit_conv(nc, w2_s, hslots[n % HR], ps)
            return ps

        def finish(n, ps):
            xsrc = xtiles[n]
            for cob in range(CB):
                ot = opool.tile([P, H * W], F32, tag="ot", name=f"ot_{n}_{cob}")
                x3 = xsrc[:, cob].rearrange("p (r c) -> p r c", c=WP)
                for half in range(2):
                    pv = ps[cob, half].rearrange("p (r c) -> p r c", c=W)
                    res = respool.tile([P, HALF], F32, tag="res", name=f"res_{n}_{cob}_{half}")
                    rv = res.rearrange("p (r c) -> p r c", c=W)
                    # res = conv2*inv2 + x   (vector engine, fused)
                    nc.vector.scalar_tensor_tensor(
                        rv[:],
                        pv[:],
                        bnv(2, cob),
                        x3[:, half * 16 + 1 : half * 16 + 17, 1 : W + 1],
                        op0=mybir.AluOpType.mult,
                        op1=mybir.AluOpType.add,
                    )
                    # out = relu(res + b2')   (scalar engine)
                    nc.scalar.activation(
                        ot[:, half * HALF : (half + 1) * HALF],
                        res[:],
                        mybir.ActivationFunctionType.Relu,
                        bias=bnv(3, cob),
                        scale=1.0,
                    )
                nc.sync.dma_start(
                    y_d[n, cob * P : (cob + 1) * P].rearrange("c h w -> c (h w)"),
                    ot[:],
                )
            del xtiles[n]

        # ---- software-pipelined emission ----
        # PE program order: conv1(0), conv1(1), conv2(0), conv1(2), conv2(1), ...
        # so the tensor engine always has conv1(n+1) to run while bn1(n)
        # completes on the scalar engine.
        for n in range(min(2, nimg)):
            load_x(n)
        ps1 = {0: conv1(0)} if nimg > 0 else {}
        for n in range(nimg):
            bn1_relu(n, ps1.pop(n))
            if n + 1 < nimg:
                ps1[n + 1] = conv1(n + 1)
            ps2 = conv2(n)
            finish(n, ps2)
            if n + 2 < nimg:
                load_x(n + 2)

    nc.compile()
    return nc


_NC_CACHE: dict = {}


def _get_nc(nimg: int = NIMG):
    if nimg not in _NC_CACHE:
        _NC_CACHE[nimg] = build(nimg)
    return _NC_CACHE[nimg]


def _prep_host(w1, g1, b1, rm1, rv1, w2, g2, b2, rm2, rv2):
    eps = 1e-5
    f = np.float32
    inv1 = (np.asarray(g1, f) / np.sqrt(np.asarray(rv1, f) + eps)).astype(f)
    b1p = (np.asarray(b1, f) - np.asarray(rm1, f) * inv1).astype(f)
    inv2 = (np.asarray(g2, f) / np.sqrt(np.asarray(rv2, f) + eps)).astype(f)
    b2p = (np.asarray(b2, f) - np.asarray(rm2, f) * inv2).astype(f)
    bnv = np.zeros((P, 4 * CB), f)
    for vi, v in enumerate([inv1, b1p, inv2, b2p]):
        for cob in range(CB):
            bnv[:, vi * CB + cob] = v[cob * P : (cob + 1) * P]

    def wt(w):
        w = np.asarray(w, f).reshape(CB, P, CB, P, 3, 3)  # [cob, co, cib, ci, ky, kx]
        w = w.transpose(2, 3, 4, 5, 0, 1)                 # [cib, ci, ky, kx, cob, co]
        return np.ascontiguousarray(w.reshape(CB, P, TAPS * CB * P))

    return wt(w1), wt(w2), bnv


def _pad_x(x):
    """[N, C, H, W] f32 -> [N, CB, 128, 34*34] with zero borders."""
    n = x.shape[0]
    xp = np.zeros((n, C, HP, WP), np.float32)
    xp[:, :, 1 : H + 1, 1 : W + 1] = x
    return np.ascontiguousarray(xp.reshape(n, CB, P, PAD))


def make_in_maps(x, w1, g1, b1, rm1, rv1, w2, g2, b2, rm2, rv2):
    x = np.asarray(x, np.float32)
    w1t, w2t, bnv = _prep_host(w1, g1, b1, rm1, rv1, w2, g2, b2, rm2, rv2)
    return [
        {
            "xp": _pad_x(x[c * NIMG : (c + 1) * NIMG]),
            "w1t": w1t,
            "w2t": w2t,
            "bnv": bnv,
        }
        for c in range(N_CORES)
    ]


def kernel(x, w1, g1, b1, rm1, rv1, w2, g2, b2, rm2, rv2):
    nc = _get_nc()
    in_maps = make_in_maps(x, w1, g1, b1, rm1, rv1, w2, g2, b2, rm2, rv2)
    res = run_bass_kernel_spmd(nc, in_maps, list(range(N_CORES)))
    return np.ascontiguousarray(
        np.concatenate([res.results[c]["y"] for c in range(N_CORES)], axis=0)
    )


# revision 14
# speedup vs baseline: 1.1489x; 1.1094x over previous
"""Trainium2 Bass kernel for a ResNet BasicBlock with ternary 3x3 convs.

  y = relu(x + bn2(conv2(relu(bn1(conv1(x))))))

x: [64, 256, 32, 32] f32.  Data-parallel over batch: 8 images per core on
8 NeuronCores.  Each 3x3 conv is computed as 9 shifted matmuls (one per tap)
accumulated in PSUM (fp32), contracting over input channels (2 blocks of
128).  Activations live in SBUF in a zero-padded 34x34 per-image layout so
every tap is a strided window read — no edge fixups.  x is pre-padded on the
host so its loads are contiguous DMAs.

Matmul dtype: fp16 by default.  The ternary weights are exact in fp16, and
fp16's 10-bit mantissa equals TF32's, so accuracy matches float32r while
weight loads get FWL (fp32 is excluded) and activation DMA traffic halves.
BN is folded into a per-channel scale+bias applied by the scalar engine
fused with ReLU; the residual add is fused into one vector-engine
scalar_tensor_tensor op.
"""

import os
from contextlib import ExitStack

import numpy as np

import concourse.bass as bass
import concourse.tile as tile
from concourse import bacc, mybir
from concourse.bass_utils import run_bass_kernel_spmd

F32 = mybir.dt.float32
F32R = mybir.dt.float32r
F16 = mybir.dt.float16
BF16 = mybir.dt.bfloat16

N_CORES = 8
N_FULL = 64          # full batch
C = 256              # channels
H = W = 32
P = 128              # partitions
CB = C // P          # channel blocks (2)
TAPS = 9
HP = H + 2           # padded height (34)
WP = W + 2           # padded width (34)
PAD = HP * WP        # padded image size (1156)
HALF = (H // 2) * W  # 512 output elements per psum tile
NIMG = N_FULL // N_CORES  # images per core (8)

# matmul dtype mode:
#   "f16"  — fp16 storage+matmul (TF32-equal mantissa, FWL weight loads)
#   "bf16" — bf16 storage+matmul (8-bit mantissa)
#   "f32r" — fp32 storage, TF32-rate matmul via float32r bitcast
#   "f32"  — exact fp32 matmul at 1/4 rate
MM_MODE = os.environ.get("BB_MM_MODE", "f16")
_BITCAST_MODE = MM_MODE in ("f32r", "f32")
# storage dtype for activations/weights
ACT_DT = {"f16": F16, "bf16": BF16, "f32r": F32, "f32": F32}[MM_MODE]
ACT_NP = {"f16": np.float16, "bf16": None, "f32r": np.float32, "f32": np.float32}[
    MM_MODE
]
_MM_DT = {"f16": F16, "bf16": BF16, "f32r": F32R, "f32": F32}[MM_MODE]

XR = 3  # x tiles in flight (conv1 input + residual)
HR = 2  # h-pad ring depth


def _mm(ap):
    """View an activation/weight AP with the matmul dtype."""
    return ap.bitcast(_MM_DT) if _BITCAST_MODE else ap


def _emit_conv(nc, ws, src, ps):
    """Emit the 72 matmuls of one 3x3 conv for one image.

    ws: per-cib weight tiles [128ci, TAPS*CB*128co]
    src: padded activation tile [128, CB, PAD]
    ps: dict (cob, half) -> PSUM tile [128, HALF]
    """
    for cib in range(CB):
        s3 = src[:, cib].rearrange("p (r c) -> p r c", c=WP)
        for tap in range(TAPS):
            dy, dx = divmod(tap, 3)
            for cob in range(CB):
                w_ap = ws[cib][:, (tap * CB + cob) * P : (tap * CB + cob + 1) * P]
                for half in range(2):
                    rhs = s3[:, half * 16 + dy : half * 16 + dy + 16, dx : dx + 32]
                    nc.tensor.matmul(
                        ps[cob, half][:],
                        _mm(w_ap),
                        _mm(rhs),
                        start=(cib == 0 and tap == 0),
                        stop=(cib == CB - 1 and tap == TAPS - 1),
                    )


def build(nimg: int = NIMG) -> bacc.Bacc:
    nc = bacc.Bacc("TRN2", target_bir_lowering=False, debug=False, enable_asserts=True)

    # x arrives host-pre-padded (and pre-cast): [nimg, CB, 128, 34*34], zero borders
    x_d = nc.dram_tensor("xp", [nimg, CB, P, PAD], ACT_DT, kind="ExternalInput")
    w1_d = nc.dram_tensor("w1t", [CB, P, TAPS * CB * P], ACT_DT, kind="ExternalInput")
    w2_d = nc.dram_tensor("w2t", [CB, P, TAPS * CB * P], ACT_DT, kind="ExternalInput")
    # bnv free layout: vec*CB + cob for vec in (inv1, b1', inv2, b2')
    bn_d = nc.dram_tensor("bnv", [P, 4 * CB], F32, kind="ExternalInput")
    y_d = nc.dram_tensor("y", [nimg, C, H, W], F32, kind="ExternalOutput")

    with tile.TileContext(nc) as tc, ExitStack() as ctx:
        wpool = ctx.enter_context(tc.tile_pool(name="weights", bufs=1))
        xpool = ctx.enter_context(tc.tile_pool(name="xpad", bufs=XR))
        hpool = ctx.enter_context(tc.tile_pool(name="hpad", bufs=1))
        pspool = ctx.enter_context(tc.tile_pool(name="psum", bufs=8, space="PSUM"))
        respool = ctx.enter_context(tc.tile_pool(name="res", bufs=4))
        opool = ctx.enter_context(tc.tile_pool(name="out", bufs=3))

        # ---- constants: weights + folded BN vectors ----
        # Weight/bn loads go on the scalar engine's HWDGE queue so they run in
        # parallel with the x loads on the sync engine's queue.  conv1 only
        # needs w1, so load it first.
        w1_s = []
        w2_s = []
        for cib in range(CB):
            t1 = wpool.tile([P, TAPS * CB * P], ACT_DT, tag=f"w1_{cib}", name=f"w1_{cib}")
            nc.scalar.dma_start(_mm(t1[:]), _mm(w1_d[cib]))
            w1_s.append(t1)
        bn_s = wpool.tile([P, 4 * CB], F32, tag="bn", name="bn_s")
        nc.scalar.dma_start(bn_s[:], bn_d[:])
        for cib in range(CB):
            t2 = wpool.tile([P, TAPS * CB * P], ACT_DT, tag=f"w2_{cib}", name=f"w2_{cib}")
            nc.scalar.dma_start(_mm(t2[:]), _mm(w2_d[cib]))
            w2_s.append(t2)

        def bnv(vec: int, cob: int):
            return bn_s[:, vec * CB + cob : vec * CB + cob + 1]

        # ---- persistent padded h ring (pads zeroed once, stay zero) ----
        hslots = [
            hpool.tile([P, CB, PAD], ACT_DT, tag=f"hp{i}", name=f"hp{i}")
            for i in range(HR)
        ]
        for s in hslots:
            for cib in range(CB):
                h3 = s[:, cib].rearrange("p (r c) -> p r c", c=WP)
                nc.vector.memset(h3[:, 0 : HP : HP - 1, :], 0.0)  # top+bottom rows
                nc.vector.memset(h3[:, 1 : HP - 1, 0 : WP : WP - 1], 0.0)  # side cols

        xtiles = {}

        def load_x(n):
            t = xpool.tile([P, CB, PAD], ACT_DT, tag="xp", name=f"xt_{n}")
            for cib in range(CB):
                nc.sync.dma_start(_mm(t[:, cib]), _mm(x_d[n, cib]))
            xtiles[n] = t

        def conv1(n):
            ps = {}
            for cob in range(CB):
                for half in range(2):
                    ps[cob, half] = pspool.tile(
                        [P, HALF], F32, tag="ps", name=f"ps1_{n}_{cob}_{half}"
                    )
            _emit_conv(nc, w1_s, xtiles[n], ps)
            return ps

        def bn1_relu(n, ps):
            hdst = hslots[n % HR]
            for cob in range(CB):
                h3 = hdst[:, cob].rearrange("p (r c) -> p r c", c=WP)
                for half in range(2):
                    pv = ps[cob, half].rearrange("p (r c) -> p r c", c=W)
                    nc.scalar.activation(
                        _mm(h3[:, half * 16 + 1 : half * 16 + 17, 1 : W + 1]),
                        pv[:],
                        mybir.ActivationFunctionType.Relu,
                        bias=bnv(1, cob),
                        scale=bnv(0, cob),
                    )

        def conv2(n):
            ps = {}
            for cob in range(CB):
                for half in range(2):
                    ps[cob, half] = pspool.tile(
                        [P, HALF], F32, tag="ps", name=f"ps2_{n}_{cob}_{half}"
                    )
            _emit_conv(nc, w2_s, hslots[n % HR], ps)
            return ps

        def finish(n, ps):
            xsrc = xtiles[n]
            for cob in range(CB):
                ot = opool.tile([P, H * W], F32, tag="ot", name=f"ot_{n}_{cob}")
                x3 = xsrc[:, cob].rearrange("p (r c) -> p r c", c=WP)
                for half in range(2):
                    pv = ps[cob, half].rearrange("p (r c) -> p r c", c=W)
                    res = respool.tile(
                        [P, HALF], F32, tag="res", name=f"res_{n}_{cob}_{half}"
                    )
                    rv = res.rearrange("p (r c) -> p r c", c=W)
                    # res = conv2*inv2 + x   (vector engine, fused)
                    nc.vector.scalar_tensor_tensor(
                        rv[:],
                        pv[:],
                        bnv(2, cob),
                        x3[:, half * 16 + 1 : half * 16 + 17, 1 : W + 1],
                        op0=mybir.AluOpType.mult,
                        op1=mybir.AluOpType.add,
                    )
                    # out = relu(res + b2')   (scalar engine)
                    nc.scalar.activation(
                        ot[:, half * HALF : (half + 1) * HALF],
                        res[:],
                        mybir.ActivationFunctionType.Relu,
                        bias=bnv(3, cob),
                        scale=1.0,
                    )
                nc.sync.dma_start(
                    y_d[n, cob * P : (cob + 1) * P].rearrange("c h w -> c (h w)"),
                    ot[:],
                )
            del xtiles[n]

        # ---- software-pipelined emission ----
        # PE program order: conv1(0), conv1(1), conv2(0), conv1(2), conv2(1), ...
        # so the tensor engine always has conv1(n+1) to run while bn1(n)
        # completes on the scalar engine.
        for n in range(min(2, nimg)):
            load_x(n)
        ps1 = {0: conv1(0)} if nimg > 0 else {}
        for n in range(nimg):
            bn1_relu(n, ps1.pop(n))
            if n + 1 < nimg:
                ps1[n + 1] = conv1(n + 1)
            ps2 = conv2(n)
            finish(n, ps2)
            if n + 2 < nimg:
                load_x(n + 2)

    nc.compile()
    return nc


_NC_CACHE: dict = {}


def _get_nc(nimg: int = NIMG):
    if nimg not in _NC_CACHE:
        _NC_CACHE[nimg] = build(nimg)
    return _NC_CACHE[nimg]


def _act_np_cast(a):
    if MM_MODE == "bf16":
        import ml_dtypes

        return a.astype(ml_dtypes.bfloat16)
    return a.astype(ACT_NP)


def _prep_host(w1, g1, b1, rm1, rv1, w2, g2, b2, rm2, rv2):
    eps = 1e-5
    f = np.float32
    inv1 = (np.asarray(g1, f) / np.sqrt(np.asarray(rv1, f) + eps)).astype(f)
    b1p = (np.asarray(b1, f) - np.asarray(rm1, f) * inv1).astype(f)
    inv2 = (np.asarray(g2, f) / np.sqrt(np.asarray(rv2, f) + eps)).astype(f)
    b2p = (np.asarray(b2, f) - np.asarray(rm2, f) * inv2).astype(f)
    bnv = np.zeros((P, 4 * CB), f)
    for vi, v in enumerate([inv1, b1p, inv2, b2p]):
        for cob in range(CB):
            bnv[:, vi * CB + cob] = v[cob * P : (cob + 1) * P]

    def wt(w):
        w = np.asarray(w, f).reshape(CB, P, CB, P, 3, 3)  # [cob, co, cib, ci, ky, kx]
        w = w.transpose(2, 3, 4, 5, 0, 1)                 # [cib, ci, ky, kx, cob, co]
        return np.ascontiguousarray(_act_np_cast(w.reshape(CB, P, TAPS * CB * P)))

    return wt(w1), wt(w2), bnv


def _pad_x(x):
    """[N, C, H, W] f32 -> [N, CB, 128, 34*34] in ACT dtype with zero borders."""
    n = x.shape[0]
    xp = np.zeros((n, C, HP, WP), np.float32)
    xp[:, :, 1 : H + 1, 1 : W + 1] = x
    return np.ascontiguousarray(_act_np_cast(xp.reshape(n, CB, P, PAD)))


def make_in_maps(x, w1, g1, b1, rm1, rv1, w2, g2, b2, rm2, rv2):
    x = np.asarray(x, np.float32)
    w1t, w2t, bnv = _prep_host(w1, g1, b1, rm1, rv1, w2, g2, b2, rm2, rv2)
    return [
        {
            "xp": _pad_x(x[c * NIMG : (c + 1) * NIMG]),
            "w1t": w1t,
            "w2t": w2t,
            "bnv": bnv,
        }
        for c in range(N_CORES)
    ]


def kernel(x, w1, g1, b1, rm1, rv1, w2, g2, b2, rm2, rv2):
    nc = _get_nc()
    in_maps = make_in_maps(x, w1, g1, b1, rm1, rv1, w2, g2, b2, rm2, rv2)
    res = run_bass_kernel_spmd(nc, in_maps, list(range(N_CORES)))
    return np.ascontiguousarray(
        np.concatenate([res.results[c]["y"] for c in range(N_CORES)], axis=0)
    )
